# revision 1
# baseline (speedup 1.0000x reference)
"""Multi-head causal attention Bass/Tile kernel for TRN2.

Per-core program (SPMD across 8 cores): each core handles one batch b and
half the heads (HPC=8). Inputs arrive pre-transposed/sliced from the host:
  xqT, xkT, xvT : [D, S]   (activations, transposed, bf16)
  wq, wk, wv    : [D, HPC*DK]  (per-core head slice, head-major columns, bf16)
  bqp, bkp      : [2*DK, HPC//2]  (bias per head-pair column)
  wo            : [HPC*DK, DO]  (slice of Wo rows for these heads, bf16)
Output: out [S, DO] = normalized-attention context @ wo  (no bo; host adds
bo + bv@Wo and sums the two head-half partials).

Layout strategy: everything transposed so softmax denominators come from a
ones-column augmented V (65th row of the AV matmul output) and attention
probabilities never need transposing.
"""

from contextlib import ExitStack

import numpy as np

import concourse.bass as bass
import concourse.mybir as mybir
import concourse.tile as tile
from concourse import library_config

F32 = mybir.dt.float32
F32R = mybir.dt.float32r
BF16 = mybir.dt.bfloat16
AF = mybir.ActivationFunctionType


def split_multiwaits(nc):
    """This walrus build accepts at most one sync-wait per instruction;
    hoist extra waits onto NOPs placed just before the instruction."""
    n_split = 0
    for fn in nc.m.functions:
        for blk in fn.blocks:
            insts = list(blk.instructions)
            out = []
            for inst in insts:
                si = inst.sync_info
                if si is not None and si.on_wait is not None and len(si.on_wait) > 1:
                    waits = list(si.on_wait)
                    for j, w in enumerate(waits[:-1]):
                        nop = mybir.InstNoOp(name=f"{inst.name}-sw{j}", ins=[], outs=[])
                        nop.engine = inst.engine
                        nop.sync_info = mybir.SyncInfo(on_wait=[w], on_update=[])
                        out.append(nop)
                    inst.sync_info = mybir.SyncInfo(
                        on_wait=[waits[-1]], on_update=list(si.on_update or [])
                    )
                    n_split += 1
                out.append(inst)
            if len(out) != len(insts):
                blk.instructions.clear()
                blk.instructions.extend(out)
    return n_split


def build(S=2048, D=1024, HPC=8, DK=64, DO=1024, QB=1024, QC=512, scale=0.125,
          mask_engine="vector", split=True, repeat=1, phases=("proj", "att", "out")):
    """Build the per-core Bass module. Returns nc."""
    assert QB % QC == 0 and S % QB == 0 and D % 128 == 0 and QC % 128 == 0
    n_dt = D // 128          # D tiles (contraction)
    n_sc = S // QC           # proj col chunks
    n_st = S // 128          # sequence tiles of 128 (key tiles)
    n_qb = S // QB           # q blocks
    n_pairs = HPC // 2
    HD = HPC * DK            # local head-concat dim
    assert HD % 128 == 0
    n_ht = HD // 128         # ctx_stack tiles
    VA = DK + 1              # V augmented with ones column

    DTQK = BF16
    DTA = BF16
    nc = bass.Bass("TRN2", target_bir_lowering=False, debug=False)

    xqT = nc.dram_tensor("xqT", [D, S], BF16, kind="ExternalInput").ap()
    xkT = nc.dram_tensor("xkT", [D, S], BF16, kind="ExternalInput").ap()
    xvT = nc.dram_tensor("xvT", [D, S], BF16, kind="ExternalInput").ap()
    wq = nc.dram_tensor("wq", [D, HD], BF16, kind="ExternalInput").ap()
    wk = nc.dram_tensor("wk", [D, HD], BF16, kind="ExternalInput").ap()
    wv = nc.dram_tensor("wv", [D, HD], BF16, kind="ExternalInput").ap()
    bqp = nc.dram_tensor("bqp", [2 * DK, n_pairs], F32, kind="ExternalInput").ap()
    bkp = nc.dram_tensor("bkp", [2 * DK, n_pairs], F32, kind="ExternalInput").ap()
    wo = nc.dram_tensor("wo", [HD, DO], BF16, kind="ExternalInput").ap()
    masks_in = nc.dram_tensor("masks_in", [128, 128], DTA,
                              kind="ExternalInput").ap()
    vones = nc.dram_tensor("vones", [128, HPC, 1], DTA, kind="ExternalInput").ap()
    out = nc.dram_tensor("out", [S, DO], F32, kind="ExternalOutput").ap()

    with tile.TileContext(nc) as tc:
      for rep in range(repeat):
        R = f"r{rep}_"
        ctx = ExitStack()
        # ---- persistent pools (live across this repeat) ----
        qk_pool = ctx.enter_context(tc.tile_pool(name=R+"qk", bufs=1))
        va_pool = ctx.enter_context(tc.tile_pool(name=R+"va", bufs=1))
        cs_pool = ctx.enter_context(tc.tile_pool(name=R+"cs", bufs=1))
        small_pool = ctx.enter_context(tc.tile_pool(name=R+"small", bufs=1))
        wo_pool = ctx.enter_context(tc.tile_pool(name=R+"wo", bufs=1))

        qt_sb = [qk_pool.tile([2 * DK, S], DTQK, name=R+f"qt{p}", tag=f"qt{p}") for p in range(n_pairs)]
        kt_sb = [qk_pool.tile([2 * DK, S], DTQK, name=R+f"kt{p}", tag=f"kt{p}") for p in range(n_pairs)]
        v_aug = [va_pool.tile([128, HPC * VA], DTA, name=R+f"va{t}", tag=f"va{t}") for t in range(n_st)]
        ctx_stack = [cs_pool.tile([128, S], BF16, name=R+f"cs{t}", tag=f"cs{t}") for t in range(n_ht)]

        bq_sb = small_pool.tile([2 * DK, n_pairs], F32, tag="bq")
        bk_sb = small_pool.tile([2 * DK, n_pairs], F32, tag="bk")
        mask_sb = small_pool.tile([128, 128], DTA, tag="mask")
        warm_sb = small_pool.tile([128, 2], F32, tag="warm")

        vones_sb = small_pool.tile([128, HPC], DTA, tag="vones")
        nc.sync.dma_start(vones_sb[:], vones[:, :, 0])
        # preload the Exp activation table off the critical path (the first
        # real exp otherwise pays a 1.3us ACT_TABLE_LOAD mid-pipeline)
        nc.scalar.activation(warm_sb[:, 0:1], vones_sb[:, 0:1], AF.Exp, scale=1.0)

        wo_t = [wo_pool.tile([128, DO], BF16, name=R+f"wo{t}", tag=f"wo{t}")
                for t in range(n_ht)]

        # ---- projections: V first (so attention can start right after Q/K
        # finish with no V-proj DMA bubble at the transition) ----
        with tc.tile_pool(name=R+"w", bufs=1) as wpool, \
             tc.tile_pool(name=R+"xin", bufs=2 * n_dt + 4) as xpool, \
             tc.tile_pool(name=R+"ppj", bufs=1, space="PSUM") as ppj:

            if "proj" in phases:
                # V inputs first (first matmuls need wv[d0] + xv[0]); smalls
                # and wo are deferred so they don't delay the first matmul
                # interleave w/x loads on two DGE queues (SP + ACT): the first
                # matmul needs only wv[d0]+xv[0], so don't serialize 16 issues
                # on one queue ahead of it
                wv_sb = wpool.tile([128, n_dt * HD], BF16, tag="wv")
                xv_t = []
                for d in range(n_dt):
                    nc.sync.dma_start(wv_sb[:, d * HD:(d + 1) * HD],
                                      wv[d * 128:(d + 1) * 128, :])
                    xt = xpool.tile([128, S], BF16, name=R+f"xv_{d}", tag="x")
                    # column-halves: V proj's first (sc, stl) groups only read
                    # the low columns, so release them early (subtile deps);
                    # alternate DGE queues so neither serializes the pacing
                    dq = nc.scalar if d % 2 == 0 else nc.gpsimd
                    dq.dma_start(xt[:, 0:S // 2],
                                 xvT[d * 128:(d + 1) * 128, 0:S // 2])
                    xv_t.append(xt)
                for d in range(n_dt):
                    dq = nc.scalar if d % 2 == 0 else nc.gpsimd
                    dq.dma_start(xv_t[d][:, S // 2:S],
                                 xvT[d * 128:(d + 1) * 128, S // 2:S])
                nc.sync.dma_start(bq_sb[:], bqp[:])
                nc.sync.dma_start(bk_sb[:], bkp[:])
                nc.sync.dma_start(mask_sb[:], masks_in[:])

                # V projection -> v_aug [128, HPC*(DK+1)]
                for sc in range(n_sc):
                    for stl in range(QC // 128):
                        st = sc * (QC // 128) + stl
                        ps = ppj.tile([128, HD], F32, tag=f"pj{stl}")
                        for d in range(n_dt):
                            nc.tensor.matmul(
                                ps[:],
                                xv_t[d][:, sc * QC + stl * 128:sc * QC + (stl + 1) * 128],
                                wv_sb[:, d * HD:(d + 1) * HD],
                                start=(d == 0), stop=(d == n_dt - 1))
                        va = v_aug[st]
                        va3 = va[:].rearrange("p (h c) -> p h c", c=VA)
                        nc.vector.tensor_copy(
                            va3[:, :, DK:VA],
                            vones_sb[:].rearrange("p h -> p h ()"))
                        nc.vector.tensor_copy(
                            va3[:, :, 0:DK], ps[:].rearrange("p (h c) -> p h c", c=DK))

                # Q and K projections (transposed: qt/kt [2DK, S])
                for (xT, w, b_sb, dst, nm) in ((xqT, wq, bq_sb, qt_sb, "q"),
                                               (xkT, wk, bk_sb, kt_sb, "k")):
                    w_sb = wpool.tile([128, n_dt * HD], BF16, tag=f"w{nm}")
                    x_t = []
                    for d in range(n_dt):
                        nc.sync.dma_start(w_sb[:, d * HD:(d + 1) * HD],
                                          w[d * 128:(d + 1) * 128, :])
                        xt = xpool.tile([128, S], BF16, name=R+f"x{nm}_{d}", tag="x")
                        nc.scalar.dma_start(xt[:], xT[d * 128:(d + 1) * 128, :])
                        x_t.append(xt)
                    if nm == "k":
                        # wo prefetch: DMA queue has slack here; first use is
                        # the qb1 outproj interleaved into qb0's attention
                        for t in range(n_ht):
                            nc.sync.dma_start(wo_t[t][:],
                                              wo[t * 128:(t + 1) * 128, :])
                    # p-outer, d-mid, sc-inner: 4 consecutive matmuls share the
                    # same stationary operand (weight chunk).  The last K group
                    # gets its own PSUM tags so attention's first QK (which
                    # reuses the pj0-3 bank addresses) needn't wait for this
                    # group's bias-add drain.
                    for p in range(n_pairs):
                        tb = 4 if (nm == "k" and p == n_pairs - 1) else 0
                        pss = [ppj.tile([128, QC], F32, name=R+f"pj_{nm}{p}{sc}",
                                        tag=f"pj{tb + sc}")
                               for sc in range(n_sc)]
                        for d in range(n_dt):
                            wchunk = w_sb[:, d * HD + p * 128:d * HD + (p + 1) * 128]
                            for sc in range(n_sc):
                                nc.tensor.matmul(
                                    pss[sc][:], wchunk,
                                    x_t[d][:, sc * QC:(sc + 1) * QC],
                                    start=(d == 0), stop=(d == n_dt - 1))
                        for sc in range(n_sc):
                            # bias-add on ACT (idle during proj; also keeps the
                            # proj->attention transition off the DVE queue)
                            nc.scalar.add(
                                dst[p][:, sc * QC:(sc + 1) * QC], pss[sc][:],
                                b_sb[:, p:p + 1])

        # ---- attention (qb-outer) with interleaved out-projection ----
        mask_eng = nc.gpsimd if mask_engine == "gpsimd" else nc.vector
        if "att" in phases:
         with tc.tile_pool(name=R+"ex", bufs=8) as expool, \
             tc.tile_pool(name=R+"rc", bufs=8) as rcpool, \
             tc.tile_pool(name=R+"dscr", bufs=8, space="DRAM") as dscrpool, \
             tc.tile_pool(name=R+"oev", bufs=3) as oev, \
             tc.tile_pool(name=R+"psc", bufs=1, space="PSUM") as psc, \
             tc.tile_pool(name=R+"pctx", bufs=1, space="PSUM") as pctx:

            NCK = min(512, DO)

            def evac_and_normalize(h, qb, ctx_ps, use_act=False):
                # evacuate ctx (unnormalized) into ctx_stack + denom row, then
                # normalize in place once the PE-free recip chain lands.
                # use_act routes the PSUM-evac copies to the scalar engine --
                # used for the last pair of a qb, where the DVE queue backlog
                # would otherwise delay the PSUM release (PE gap -> HAM cold).
                t, row0 = h // 2, (h % 2) * DK
                cs_slice = ctx_stack[t][row0:row0 + DK, qb * QB:(qb + 1) * QB]
                FQ = QB // 128
                dn = rcpool.tile([1, QB], F32, name=R + f"dn{h}_{qb}", tag="dn")
                # dn on ACT (tiny, starts the recip chain fast); cs on DVE
                # except for the last pair where the DVE backlog would delay
                # the PSUM release (the next pair's AV + PE warmth gate on it)
                nc.scalar.copy(dn[:], ctx_ps[DK:DK + 1, :])
                if use_act:
                    nc.scalar.copy(cs_slice, ctx_ps[0:DK, :])
                else:
                    nc.vector.tensor_copy(cs_slice, ctx_ps[0:DK, :])
                # last pair's chain goes via the (tail-idle) SP queue so the
                # two heads' chains don't serialize behind one DGE queue
                dq = nc.sync if use_act else nc.gpsimd
                ds1 = dscrpool.tile([1, QB], F32, name=R + f"ds1_{h}_{qb}", tag="ds1")
                dq.dma_start(ds1[:], dn[:])
                dnp = rcpool.tile([128, FQ], F32, name=R + f"dnp{h}_{qb}", tag="dnp")
                dq.dma_start(dnp[:], ds1[0, :].rearrange("(p f) -> p f", f=FQ))
                rcp = rcpool.tile([128, FQ], F32R, name=R + f"rcp{h}_{qb}", tag="rcp")
                with nc.allow_low_precision(reason="denom recip"):
                    nc.vector.reciprocal(rcp[:], dnp[:])
                ds2 = dscrpool.tile([1, QB], F32R, name=R + f"ds2_{h}_{qb}", tag="ds2")
                dq.dma_start(ds2[0, :].rearrange("(p f) -> p f", f=FQ), rcp[:])
                bc_sb = rcpool.tile([128, QB], F32R, name=R + f"bc{h}_{qb}", tag="bc")
                dq.dma_start(bc_sb[row0:row0 + DK, :],
                             ds2[:].broadcast_to([DK, QB]))
                nc.vector.tensor_mul(cs_slice, cs_slice, bc_sb[row0:row0 + DK, :])

            def outproj(qb, stls):
                # project finished ctx_stack columns (q rows qb*QB..) through
                # wo for the given stl subset; spread across the next qb's
                # pair streams so the PSUM/DVE load amortizes.
                for stl in stls:
                    st = qb * (QB // 128) + stl
                    ps = psc.tile([128, QB], F32, name=R + f"op{qb}_{stl}",
                                  tag=("sc_e" if stl % 2 == 0 else "sc_o"))
                    for t in range(n_ht):
                        lhsT = ctx_stack[t][:, st * 128:(st + 1) * 128]
                        for nck in range(DO // NCK):
                            nc.tensor.matmul(ps[:, nck * NCK:(nck + 1) * NCK],
                                             lhsT,
                                             wo_t[t][:, nck * NCK:(nck + 1) * NCK],
                                             start=(t == 0), stop=(t == n_ht - 1))
                    ev = oev.tile([128, QB], F32, name=R + f"opev{qb}_{stl}",
                                  tag="ev")
                    nc.vector.tensor_copy(ev[:], ps[:])
                    nc.sync.dma_start(out[st * 128:(st + 1) * 128, :], ev[:])

            def outproj_final(qb):
                # final out-projection: by now the ctx PSUM banks are being
                # evacuated, so rotate ps across ALL four PSUM tags (4 live
                # tiles) and defer the last pair's t-accumulation, letting
                # the t=0..2 matmuls overlap the last normalize chain.
                tags = ["sc_e", "sc_o", "ctx_e", "ctx_o"]
                pss = {}
                G = 4
                for g in range(0, QB // 128, G):
                    group = [s for s in range(g, min(QB // 128, g + G))]
                    for stl in group:
                        st = qb * (QB // 128) + stl
                        ps = psc.tile([128, QB], F32, name=R + f"of{stl}",
                                      tag=tags[stl % 2]) if stl % G < 2 else \
                             pctx.tile([128, QB], F32, name=R + f"of{stl}",
                                       tag=tags[2 + stl % 2])
                        pss[stl] = ps
                        for t in range(n_ht - 1):
                            lhsT = ctx_stack[t][:, st * 128:(st + 1) * 128]
                            for nck in range(DO // NCK):
                                nc.tensor.matmul(
                                    ps[:, nck * NCK:(nck + 1) * NCK], lhsT,
                                    wo_t[t][:, nck * NCK:(nck + 1) * NCK],
                                    start=(t == 0), stop=False)
                    for stl in group:
                        st = qb * (QB // 128) + stl
                        ps = pss[stl]
                        t = n_ht - 1
                        lhsT = ctx_stack[t][:, st * 128:(st + 1) * 128]
                        for nck in range(DO // NCK):
                            nc.tensor.matmul(
                                ps[:, nck * NCK:(nck + 1) * NCK], lhsT,
                                wo_t[t][:, nck * NCK:(nck + 1) * NCK],
                                start=False, stop=True)
                        ev = oev.tile([128, QB], F32, name=R + f"oev{stl}",
                                      tag="ev")
                        nc.vector.tensor_copy(ev[:], ps[:])
                        nc.sync.dma_start(out[st * 128:(st + 1) * 128, :], ev[:])

            pending_out = None   # outproj deferred into the next qb's stream
            qb_order = list(range(n_qb))
            for qb in qb_order:
                for p in range(n_pairs):
                    he, ho = 2 * p, 2 * p + 1
                    qt_e = qt_sb[p][0:DK, :]
                    kt_e = kt_sb[p][0:DK, :]
                    qt_o = qt_sb[p][DK:2 * DK, :]
                    kt_o = kt_sb[p][DK:2 * DK, :]
                    ktm = ((qb + 1) * QB) // 128 - 1
                    ctx_e = pctx.tile([VA, QB], F32, tag="ctx_e")
                    ctx_o = pctx.tile([VA, QB], F32, tag="ctx_o")
                    diag0 = (qb * QB + QC) // 128   # first deep-diagonal kt
                    ex2s = {}
                    pend_av = []
                    for kt in range(ktm + 1):
                        rel_lo = max(0, kt * 128 - qb * QB)
                        if rel_lo >= QC:
                            # deep-diagonal tile: single chunk of N<=512 cols.
                            # Pack BOTH parities into one 2-bank PSUM tile
                            # (e -> bank 1, o -> bank 2): one exp + one mask
                            # instruction for both, and the e/o tags then
                            # rotate at 2-kt depth.  Pass 1: QK+exp+mask only;
                            # the AVs are emitted as one dense PE burst after
                            # the loop, so the sparse per-kt chain latency
                            # doesn't stall the in-order PE queue per kt.
                            N = QB - rel_lo
                            sc2 = psc.tile([128, QB], F32, name=R + f"sc2_{qb}_{p}_{kt}",
                                           tag=("sc_e" if kt % 2 == 0 else "sc_o"))
                            nc.tensor.matmul(
                                sc2[:, 0:N], kt_e[:, kt * 128:(kt + 1) * 128],
                                qt_e[:, qb * QB + rel_lo:qb * QB + QB],
                                start=True, stop=True)
                            nc.tensor.matmul(
                                sc2[:, QC:QC + N], kt_o[:, kt * 128:(kt + 1) * 128],
                                qt_o[:, qb * QB + rel_lo:qb * QB + QB],
                                start=True, stop=True)
                            ex2 = expool.tile([128, 2, QC], DTA,
                                              name=R + f"ex2_{qb}_{p}_{kt}",
                                              tag="ex2", bufs=6)
                            sc3 = sc2[:].rearrange("q (g c) -> q g c", c=QC)
                            nc.scalar.activation(ex2[:, :, 0:N], sc3[:, :, 0:N],
                                                 AF.Exp, scale=scale)
                            mask_eng.tensor_mul(
                                ex2[:, :, 0:128], ex2[:, :, 0:128],
                                mask_sb[:].rearrange("q c -> q () c")
                                .broadcast_to([128, 2, 128]))
                            ex2s[kt] = ex2
                            continue
                        # 128-granularity causal chunks within this qb block:
                        # first chunk [rel_lo, next 512 boundary), then 512s
                        chunks = []
                        c0 = rel_lo
                        while c0 < QB:
                            c1 = min(QB, (c0 // QC + 1) * QC)
                            chunks.append((c0, c1))
                            c0 = c1
                        sc_e = psc.tile([128, QB], F32, tag="sc_e")
                        sc_o = psc.tile([128, QB], F32, tag="sc_o")
                        for (c0, c1) in chunks:
                            nc.tensor.matmul(
                                sc_e[:, c0:c1], kt_e[:, kt * 128:(kt + 1) * 128],
                                qt_e[:, qb * QB + c0:qb * QB + c1],
                                start=True, stop=True)
                            nc.tensor.matmul(
                                sc_o[:, c0:c1], kt_o[:, kt * 128:(kt + 1) * 128],
                                qt_o[:, qb * QB + c0:qb * QB + c1],
                                start=True, stop=True)
                        ex_e = expool.tile([128, QB], DTA, tag="ex_e")
                        ex_o = expool.tile([128, QB], DTA, tag="ex_o")
                        nc.scalar.activation(ex_e[:, rel_lo:QB], sc_e[:, rel_lo:QB],
                                             AF.Exp, scale=scale)
                        nc.scalar.activation(ex_o[:, rel_lo:QB], sc_o[:, rel_lo:QB],
                                             AF.Exp, scale=scale)
                        if kt * 128 >= qb * QB:
                            # diagonal tile: triangular mask on the 128 cols
                            # starting at rel_lo
                            mask_eng.tensor_mul(ex_e[:, rel_lo:rel_lo + 128],
                                                ex_e[:, rel_lo:rel_lo + 128],
                                                mask_sb[:])
                            mask_eng.tensor_mul(ex_o[:, rel_lo:rel_lo + 128],
                                                ex_o[:, rel_lo:rel_lo + 128],
                                                mask_sb[:])
                        # 1-kt software pipeline: defer this kt's AVs until the
                        # next kt's QK/exp are emitted, so the PE queue has
                        # non-AV work while the pair's first AV waits for the
                        # previous pair's ctx-bank release (pair-boundary gap)
                        pend_av.append((kt, chunks, ex_e, ex_o))
                        if len(pend_av) > 3:
                            akt, achunks, aex_e, aex_o = pend_av.pop(0)
                            for (c0, c1) in achunks:
                                last_kt = min(ktm, (qb * QB + c1) // 128 - 1)
                                nc.tensor.matmul(
                                    ctx_e[:, c0:c1],
                                    v_aug[akt][:, he * VA:(he + 1) * VA],
                                    aex_e[:, c0:c1],
                                    start=(akt == 0), stop=(akt == last_kt))
                                nc.tensor.matmul(
                                    ctx_o[:, c0:c1],
                                    v_aug[akt][:, ho * VA:(ho + 1) * VA],
                                    aex_o[:, c0:c1],
                                    start=(akt == 0), stop=(akt == last_kt))
                        if kt in (2, 6) and pending_out is not None:
                            # one stl per insertion point (kt 2 and 6 of each
                            # pair): halves how long each wedge monopolizes
                            # the score PSUM tags and the DVE evac queue
                            n_stl = QB // 128
                            per = max(1, n_stl // n_pairs)
                            base = p * per + (per // 2 if kt == 6 else 0)
                            end = (p + 1) * per if p < n_pairs - 1 else n_stl
                            outproj(pending_out,
                                    range(base, min(end, base + max(1, per // 2))
                                          if kt == 2 else end))
                    # drain the software-pipelined last standard-kt AVs
                    for (akt, achunks, aex_e, aex_o) in pend_av:
                        for (c0, c1) in achunks:
                            last_kt = min(ktm, (qb * QB + c1) // 128 - 1)
                            nc.tensor.matmul(
                                ctx_e[:, c0:c1],
                                v_aug[akt][:, he * VA:(he + 1) * VA],
                                aex_e[:, c0:c1],
                                start=(akt == 0), stop=(akt == last_kt))
                            nc.tensor.matmul(
                                ctx_o[:, c0:c1],
                                v_aug[akt][:, ho * VA:(ho + 1) * VA],
                                aex_o[:, c0:c1],
                                start=(akt == 0), stop=(akt == last_kt))
                    # pass 2: the deep-diagonal AV burst (dense back-to-back
                    # PE work; exps for these kts are already in flight)
                    for kt in range(diag0, ktm + 1):
                        rel_lo = kt * 128 - qb * QB
                        N = QB - rel_lo
                        ex2 = ex2s[kt]
                        nc.tensor.matmul(
                            ctx_e[:, rel_lo:QB], v_aug[kt][:, he * VA:(he + 1) * VA],
                            ex2[:, 0, 0:N], start=False, stop=(kt == ktm))
                        nc.tensor.matmul(
                            ctx_o[:, rel_lo:QB], v_aug[kt][:, ho * VA:(ho + 1) * VA],
                            ex2[:, 1, 0:N], start=False, stop=(kt == ktm))
                    evac_and_normalize(he, qb, ctx_e, use_act=(p == n_pairs - 1))
                    evac_and_normalize(ho, qb, ctx_o, use_act=(p == n_pairs - 1))
                pending_out = qb
            outproj_final(qb_order[-1])
        ctx.close()

    if split:
        split_multiwaits(nc)
    return nc


def core_inputs(queries, keys, values, Wq, bq, Wk, bk, Wv, bv, Wo, core, n_cores=8,
                HPC=None):
    """Host-side shard prep for one core. core -> (batch, head-group)."""
    import ml_dtypes
    B = queries.shape[0]
    H = Wq.shape[0]
    groups = n_cores // B
    b, hg = core // groups, core % groups
    if HPC is None:
        HPC = H // groups
    h0 = hg * HPC
    DK = Wq.shape[2]
    bf16 = ml_dtypes.bfloat16

    def wsel(W):
        # [H, D, dk] -> [D, HPC*dk], head-major columns
        return np.ascontiguousarray(
            W[h0:h0 + HPC].transpose(1, 0, 2).reshape(W.shape[1], HPC * DK)
        ).astype(bf16)

    def bpairs(bias):
        # [H, dk] -> [2*dk, HPC//2]
        bsel = bias[h0:h0 + HPC].reshape(HPC // 2, 2 * DK)
        return np.ascontiguousarray(bsel.T)

    x = np.arange(128)[:, None]
    y = np.arange(128)[None, :]
    mask = (y - x >= 0).astype(np.float32)
    return {
        "masks_in": mask.astype(bf16),
        "vones": np.ones((128, HPC, 1), bf16),
        "xqT": np.ascontiguousarray(queries[b].T).astype(bf16),
        "xkT": np.ascontiguousarray(keys[b].T).astype(bf16),
        "xvT": np.ascontiguousarray(values[b].T).astype(bf16),
        "wq": wsel(Wq), "wk": wsel(Wk), "wv": wsel(Wv),
        "bqp": bpairs(bq), "bkp": bpairs(bk),
        "wo": np.ascontiguousarray(Wo[h0 * DK:(h0 + HPC) * DK, :]).astype(bf16),
    }


def assemble(results, B, n_cores, bias_total):
    """Sum head-group partials per batch and add the host-side bias."""
    groups = n_cores // B
    outs = []
    for b in range(B):
        acc = results[b * groups]["out"].astype(np.float64)
        for g in range(1, groups):
            acc = acc + results[b * groups + g]["out"]
        outs.append(acc + bias_total)
    return np.stack(outs).astype(np.float32)


# ---------------------------------------------------------------------------
# Harness entry point: full (unsharded) inputs -> full output.
# Shards batch (4) x head-halves (2) across the 8 NeuronCores, runs the Bass
# kernel via run_bass_kernel_spmd, then sums head-half partials per batch on
# the host (+ bias fold: out += bo + bv @ Wo, exact because attention rows
# sum to 1 after normalization).
# ---------------------------------------------------------------------------
_CACHE = {}


def kernel(**inputs):
    from concourse.bass_utils import run_bass_kernel_spmd

    queries = np.asarray(inputs["queries"], np.float32)
    keys = np.asarray(inputs["keys"], np.float32)
    values = np.asarray(inputs["values"], np.float32)
    Wq = np.asarray(inputs["Wq"], np.float32)
    bq = np.asarray(inputs["bq"], np.float32)
    Wk = np.asarray(inputs["Wk"], np.float32)
    bk = np.asarray(inputs["bk"], np.float32)
    Wv = np.asarray(inputs["Wv"], np.float32)
    bv = np.asarray(inputs["bv"], np.float32)
    Wo = np.asarray(inputs["Wo"], np.float32)
    bo = np.asarray(inputs["bo"], np.float32)

    B = queries.shape[0]
    n_cores = 8
    if "nc" not in _CACHE:
        _CACHE["nc"] = build()
    nc = _CACHE["nc"]
    in_maps = [core_inputs(queries, keys, values, Wq, bq, Wk, bk, Wv, bv, Wo,
                           core=c, n_cores=n_cores) for c in range(n_cores)]
    res = run_bass_kernel_spmd(nc, in_maps, list(range(n_cores)))
    bias_total = bo + bv.reshape(-1) @ Wo
    return assemble(res.results, B, n_cores, bias_total)



# revision 19
# speedup vs baseline: 1.2681x; 1.2681x over previous
"""Multi-head causal attention Bass/Tile kernel for TRN2 (fp8 DoubleRow v3).

Per-core program (SPMD across 8 cores): each core handles one batch b and
half the heads (HPC=8).  Host-prepped inputs:
  xqT, xkT, xvT : [D, S]       fp8e4 (activations, transposed)
  xqTb,xkTb,xvTb: [D, 512]     bf16  (first 512 columns, for the head block)
  wq, wk        : [D, HPC*DK]  fp8e4, pre-scaled x64 (head-major columns)
  wv            : [D, HPC*DK]  fp8e4, pre-scaled x32
  wqb, wkb, wvb : [D, HPC*DK]  bf16  (unscaled)
  bqp, bkp      : [2*DK, HPC//2] f32, pre-scaled x64
  bqb, bkb      : [2*DK, HPC//2] f32, unscaled
  wo            : [HPC*DK, DO] bf16, pre-scaled /32
Output: out [S, DO] bf16 = 32 * normalized-attention context @ (Wo/32);
host adds bo + bv@Wo and sums the two head-half partials.

Precision split: causal attention rows 0..511 have few effective keys, so
fp8 noise on V/probs/scores passes straight through instead of averaging
out -- those rows (the "head block") run fully in bf16.  Rows 512+ run the
fp8 pipeline (error ~4e-3 vs the 4.5e-2 of fp8-everywhere).

PE work runs fp8 perf_mode=DoubleRow (two 128-row k-tiles per instruction)
for the three input projections and the AV matmuls; QK and the out-proj
stay bf16 (QK gains nothing from fp8: its cycle count is moving-column
bound at K=64).

Scheduling: the fp8 projections for later q-blocks are emitted as PE
filler INSIDE the earlier q-blocks' attention streams, so the scalar
engine's ~150us of exp work starts as early as possible and the PE never
sits behind a monolithic projection phase.

Layout: everything transposed so softmax denominators come from a
ones-column augmented V (65th row of the AV matmul output) and attention
probabilities never need transposing.  fp8 probabilities live in
kt-PAIR-packed tiles exa[128, par, kt&1, QB] so one AV DoubleRow matmul
consumes two key tiles.
"""

from contextlib import ExitStack

import numpy as np

import concourse.bass as bass
import concourse.mybir as mybir
import concourse.tile as tile

F32 = mybir.dt.float32
F32R = mybir.dt.float32r
BF16 = mybir.dt.bfloat16
FP8 = mybir.dt.float8e4
AF = mybir.ActivationFunctionType
DR = mybir.MatmulPerfMode.DoubleRow


def split_multiwaits(nc):
    """This walrus build accepts at most one sync-wait per instruction;
    hoist extra waits onto NOPs placed just before the instruction."""
    n_split = 0
    for fn in nc.m.functions:
        for blk in fn.blocks:
            insts = list(blk.instructions)
            out = []
            for inst in insts:
                si = inst.sync_info
                if si is not None and si.on_wait is not None and len(si.on_wait) > 1:
                    waits = list(si.on_wait)
                    for j, w in enumerate(waits[:-1]):
                        nop = mybir.InstNoOp(name=f"{inst.name}-sw{j}", ins=[], outs=[])
                        nop.engine = inst.engine
                        nop.sync_info = mybir.SyncInfo(on_wait=[w], on_update=[])
                        out.append(nop)
                    inst.sync_info = mybir.SyncInfo(
                        on_wait=[waits[-1]], on_update=list(si.on_update or [])
                    )
                    n_split += 1
                out.append(inst)
            if len(out) != len(insts):
                blk.instructions.clear()
                blk.instructions.extend(out)
    return n_split


def build(S=2048, D=1024, HPC=8, DK=64, DO=1024, QB=512, scale=0.125 / 4096):
    """Build the per-core Bass module. Returns nc."""
    n_dt = D // 128          # d tiles
    n_dp = D // 256          # d-tile PAIRS (DoubleRow contraction)
    n_st = S // 128          # key tiles of 128
    n_sp = n_st // 2         # key-tile pairs
    n_qb = S // QB           # q blocks
    n_pairs = HPC // 2       # head pairs
    HD = HPC * DK            # local head-concat dim
    n_ht = HD // 128         # ctx_stack tiles
    VA = DK + 1              # V augmented with ones column
    VAP = DK + 2             # pad: pair stride VAP*HPC must be %16
    HB = QB // 2             # AV split boundary within a q block
    FQ = QB // 128
    NHB = QB // 128          # head-block kt count (rows 0..QB-1)

    assert (VAP * HPC) % 16 == 0

    nc = bass.Bass("TRN2", target_bir_lowering=False, debug=False)

    # all inputs arrive host-prepacked in their exact SBUF layouts so each
    # loads with a single DMA (DGE issue time on the queue engines is ~600ns
    # per descriptor -- 100 small loads would cost ~60us of queue time)
    xqT = nc.dram_tensor("xqT", [128, n_dp, 2, S - QB], FP8, kind="ExternalInput").ap()
    xkT = nc.dram_tensor("xkT", [128, n_dp, 2, S - QB], FP8, kind="ExternalInput").ap()
    xvT = nc.dram_tensor("xvT", [128, n_dp, 2, S], FP8, kind="ExternalInput").ap()
    xqTb = nc.dram_tensor("xqTb", [128, n_dt, QB], BF16, kind="ExternalInput").ap()
    xkTb = nc.dram_tensor("xkTb", [128, n_dt, QB], BF16, kind="ExternalInput").ap()
    xvTb = nc.dram_tensor("xvTb", [128, n_dt, QB], BF16, kind="ExternalInput").ap()
    wq = nc.dram_tensor("wq", [128, n_dp, 2, HD], FP8, kind="ExternalInput").ap()
    wk = nc.dram_tensor("wk", [128, n_dp, 2, HD], FP8, kind="ExternalInput").ap()
    wv = nc.dram_tensor("wv", [128, n_dp, 2, HD], FP8, kind="ExternalInput").ap()
    wqb = nc.dram_tensor("wqb", [128, n_dt, HD], BF16, kind="ExternalInput").ap()
    wkb = nc.dram_tensor("wkb", [128, n_dt, HD], BF16, kind="ExternalInput").ap()
    wvb = nc.dram_tensor("wvb", [128, n_dt, HD], BF16, kind="ExternalInput").ap()
    bqp = nc.dram_tensor("bqp", [2 * DK, n_pairs], F32, kind="ExternalInput").ap()
    bkp = nc.dram_tensor("bkp", [2 * DK, n_pairs], F32, kind="ExternalInput").ap()
    bqb = nc.dram_tensor("bqb", [2 * DK, n_pairs], F32, kind="ExternalInput").ap()
    bkb = nc.dram_tensor("bkb", [2 * DK, n_pairs], F32, kind="ExternalInput").ap()
    wo = nc.dram_tensor("wo", [128, n_ht, DO], BF16, kind="ExternalInput").ap()
    # [128, 256] bf16: cols 0-127 all-zero, cols 128-255 lower-tri ones
    masks_in = nc.dram_tensor("masks_in", [128, 256], BF16,
                              kind="ExternalInput").ap()
    vones = nc.dram_tensor("vones", [128, HPC, 1], FP8, kind="ExternalInput").ap()
    vonesb = nc.dram_tensor("vonesb", [128, HPC, 1], BF16,
                            kind="ExternalInput").ap()
    out = nc.dram_tensor("out", [S, DO], BF16, kind="ExternalOutput").ap()

    with tile.TileContext(nc) as tc:
        ctx = ExitStack()
        # ---- persistent pools ----
        qk_pool = ctx.enter_context(tc.tile_pool(name="qk", bufs=1))
        va_pool = ctx.enter_context(tc.tile_pool(name="va", bufs=1))
        cs_pool = ctx.enter_context(tc.tile_pool(name="cs", bufs=1))
        small_pool = ctx.enter_context(tc.tile_pool(name="small", bufs=1))
        wo_pool = ctx.enter_context(tc.tile_pool(name="wo", bufs=1))
        w8_pool = ctx.enter_context(tc.tile_pool(name="w8", bufs=1))
        x8_pool = ctx.enter_context(tc.tile_pool(name="x8", bufs=1))
        # bf16 head-block inputs live only through phases A/B
        bctx = ExitStack()
        xb_pool = bctx.enter_context(tc.tile_pool(name="xb", bufs=1))
        wb_pool = bctx.enter_context(tc.tile_pool(name="wb", bufs=1))

        qt_sb = [qk_pool.tile([2 * DK, S], BF16, name=f"qt{p}", tag=f"qt{p}")
                 for p in range(n_pairs)]
        kt_sb = [qk_pool.tile([2 * DK, S], BF16, name=f"kt{p}", tag=f"kt{p}")
                 for p in range(n_pairs)]
        # v_pair[j]: fp8 key tiles (2j, 2j+1) packed for DoubleRow AV
        v_pair = [va_pool.tile([128, 2, HPC * VAP], FP8, name=f"vp{j}", tag=f"vp{j}")
                  for j in range(n_sp)]
        # vb[st]: bf16 V for the head block (keys 0..QB-1)
        vb = [va_pool.tile([128, HPC * VA], BF16, name=f"vb{t}", tag=f"vb{t}")
              for t in range(NHB)]
        ctx_stack = [cs_pool.tile([128, S], BF16, name=f"cs{t}", tag=f"cs{t}")
                     for t in range(n_ht)]

        bq_sb = small_pool.tile([2 * DK, n_pairs], F32, tag="bq")
        bk_sb = small_pool.tile([2 * DK, n_pairs], F32, tag="bk")
        bqb_sb = small_pool.tile([2 * DK, n_pairs], F32, tag="bqb")
        bkb_sb = small_pool.tile([2 * DK, n_pairs], F32, tag="bkb")
        mask_sb = small_pool.tile([128, 256], BF16, tag="mask")
        warm_sb = small_pool.tile([128, 2], F32, tag="warm")
        vones_sb = small_pool.tile([128, HPC], FP8, tag="vones")
        vonesb_sb = small_pool.tile([128, HPC], BF16, tag="vonesb")

        wo_t = wo_pool.tile([128, n_ht, DO], BF16, tag="wo")

        # ---------------- DMA issue (front-loaded) ----------------
        # bf16 head-block inputs first (phase A starts on them), then fp8.
        nc.sync.dma_start(vonesb_sb[:], vonesb[:, :, 0])
        nc.sync.dma_start(vones_sb[:], vones[:, :, 0])
        nc.sync.dma_start(mask_sb[:], masks_in[:])
        # preload the Exp activation table off the critical path
        nc.scalar.activation(warm_sb[:, 0:1], warm_sb[:, 1:2], AF.Exp, scale=0.0)

        xb_t = {}
        for nm, src in (("v", xvTb), ("k", xkTb), ("q", xqTb)):
            xt = xb_pool.tile([128, n_dt, QB], BF16, tag=f"xb{nm}")
            for dd in range(n_dt):
                (nc.sync if dd % 2 == 0 else nc.scalar).dma_start(
                    xt[:, dd, :], src[dd * 128:(dd + 1) * 128, :])
            xb_t[nm] = xt
        wb_t = {}
        for nm, src in (("v", wvb), ("k", wkb), ("q", wqb)):
            wt = wb_pool.tile([128, n_dt, HD], BF16, tag=f"wb{nm}")
            for dd in range(n_dt):
                (nc.scalar if dd % 2 == 0 else nc.sync).dma_start(
                    wt[:, dd, :], src[dd * 128:(dd + 1) * 128, :])
            wb_t[nm] = wt
        nc.sync.dma_start(bqb_sb[:], bqb[:])
        nc.sync.dma_start(bkb_sb[:], bkb[:])
        nc.sync.dma_start(bq_sb[:], bqp[:])
        nc.sync.dma_start(bk_sb[:], bkp[:])

        # fp8 weights + activations (xq/xk only need cols QB.. for fp8 path)
        w8 = {}
        for nm, src in (("v", wv), ("k", wk), ("q", wq)):
            wt = w8_pool.tile([128, n_dp, 2, HD], FP8, tag=f"w8{nm}")
            for j in range(n_dp):
                nc.scalar.dma_start(wt[:, j, 0, :], src[(2 * j) * 128:(2 * j + 1) * 128, :])
                nc.scalar.dma_start(wt[:, j, 1, :], src[(2 * j + 1) * 128:(2 * j + 2) * 128, :])
            w8[nm] = wt
        x8 = {}
        for nm, src, c0 in (("v", xvT, 0), ("k", xkT, QB), ("q", xqT, QB)):
            xt = x8_pool.tile([128, n_dp, 2, S - c0], FP8, tag=f"x8{nm}")
            for j in range(n_dp):
                for ki in range(2):
                    dq = nc.scalar if (j + ki) % 2 == 0 else nc.sync
                    dq.dma_start(xt[:, j, ki, :],
                                 src[(2 * j + ki) * 128:(2 * j + ki + 1) * 128, c0:S])
            x8[nm] = xt
        for t in range(n_ht):
            nc.sync.dma_start(wo_t[t][:], wo[t * 128:(t + 1) * 128, :])

        # ---------------- compute ----------------
        with tc.tile_pool(name="ex", bufs=5) as expool, \
             tc.tile_pool(name="exb", bufs=4) as exbpool, \
             tc.tile_pool(name="rc", bufs=2) as rcpool, \
             tc.tile_pool(name="bc", bufs=3) as bcpool, \
             tc.tile_pool(name="dscr", bufs=8, space="DRAM") as dscrpool, \
             tc.tile_pool(name="oev", bufs=2) as oev:

            pctx2 = ExitStack()
            psc = pctx2.enter_context(tc.tile_pool(name="psc", bufs=1, space="PSUM"))
            pctx = pctx2.enter_context(tc.tile_pool(name="pctx", bufs=1, space="PSUM"))

            NCK = 512
            fill_tag = [0]

            def ftag():
                fill_tag[0] ^= 1
                return "fillA" if fill_tag[0] else "fillB"

            # ---- projection units (each: GEMM into a 1-bank fill tile
            # + evac), emitted inline or as filler inside attention ----
            def proj_v_bf16(st):
                ps = psc.tile([128, HD], F32, name=f"pvb{st}", tag=ftag())
                for dd in range(n_dt):
                    nc.tensor.matmul(ps[:], xb_t["v"][:, dd, st * 128:(st + 1) * 128],
                                     wb_t["v"][:, dd, :],
                                     start=(dd == 0), stop=(dd == n_dt - 1))
                v3 = vb[st][:].rearrange("p (h c) -> p h c", c=VA)
                nc.vector.tensor_copy(v3[:, :, DK:VA],
                                      vonesb_sb[:].rearrange("p h -> p h ()"))
                nc.vector.tensor_copy(v3[:, :, 0:DK],
                                      ps[:].rearrange("p (h c) -> p h c", c=DK))

            def proj_qk_bf16(nm, p):
                dst, b_sb = (qt_sb, bqb_sb) if nm == "q" else (kt_sb, bkb_sb)
                ps = psc.tile([128, QB], F32, name=f"pb{nm}{p}", tag=ftag())
                for dd in range(n_dt):
                    nc.tensor.matmul(
                        ps[:], wb_t[nm][:, dd, p * 128:(p + 1) * 128],
                        xb_t[nm][:, dd, :],
                        start=(dd == 0), stop=(dd == n_dt - 1))
                nc.vector.tensor_scalar_add(dst[p][:, 0:QB], ps[:], b_sb[:, p:p + 1])

            def proj_v_fp8(st):
                ps = psc.tile([128, HD], F32, name=f"pv8{st}", tag=ftag())
                for j in range(n_dp):
                    nc.tensor.matmul(
                        ps[:], x8["v"][:, j, :, st * 128:(st + 1) * 128],
                        w8["v"][:, j, :, :],
                        start=(j == 0), stop=(j == n_dp - 1), perf_mode=DR)
                vp3 = v_pair[st // 2][:, st % 2, :].rearrange(
                    "p (h c) -> p h c", c=VAP)
                nc.vector.tensor_copy(vp3[:, :, DK:DK + 1],
                                      vones_sb[:].rearrange("p h -> p h ()"))
                nc.vector.tensor_copy(vp3[:, :, 0:DK],
                                      ps[:].rearrange("p (h c) -> p h c", c=DK))

            def proj_qk_fp8(nm, p, sc):
                dst, b_sb = (qt_sb, bq_sb) if nm == "q" else (kt_sb, bk_sb)
                ps = psc.tile([128, QB], F32, name=f"p8{nm}{p}{sc}", tag=ftag())
                for j in range(n_dp):
                    nc.tensor.matmul(
                        ps[:], w8[nm][:, j, :, p * 128:(p + 1) * 128],
                        x8[nm][:, j, :, (sc - 1) * QB:sc * QB],
                        start=(j == 0), stop=(j == n_dp - 1), perf_mode=DR)
                nc.vector.tensor_scalar_add(
                    dst[p][:, sc * QB:(sc + 1) * QB], ps[:], b_sb[:, p:p + 1])

            # ---- evac + normalize, one merged recip chain per head pair
            # (both parities share the DRAM-bounce reshape/broadcast DMAs;
            # all chain hops ride the sync queue, normalize on gpsimd) ----
            def evac_pair(p, qb, ctx_e, ctx_o, use_act=False):
                he, ho = 2 * p, 2 * p + 1
                t = he // 2
                cs_e = ctx_stack[t][0:DK, qb * QB:(qb + 1) * QB]
                cs_o = ctx_stack[t][DK:2 * DK, qb * QB:(qb + 1) * QB]
                dn = rcpool.tile([1, 2 * QB], F32, name=f"dn{p}_{qb}", tag="dn")
                nc.vector.tensor_copy(dn[0:1, 0:QB], ctx_e[DK:DK + 1, :])
                nc.vector.tensor_copy(dn[0:1, QB:2 * QB], ctx_o[DK:DK + 1, :])
                if use_act:
                    nc.scalar.copy(cs_e, ctx_e[0:DK, :])
                    nc.scalar.copy(cs_o, ctx_o[0:DK, :])
                else:
                    nc.vector.tensor_copy(cs_e, ctx_e[0:DK, :])
                    nc.vector.tensor_copy(cs_o, ctx_o[0:DK, :])
                dq = nc.sync
                FH = 2 * QB // 128
                ds1 = dscrpool.tile([1, 2 * QB], F32, name=f"ds1_{p}_{qb}", tag="ds1")
                dq.dma_start(ds1[:], dn[:])
                dnp = rcpool.tile([128, FH], F32, name=f"dnp{p}_{qb}", tag="dnp")
                dq.dma_start(dnp[:], ds1[0, :].rearrange("(pp f) -> pp f", f=FH))
                rcp = rcpool.tile([128, FH], F32R, name=f"rcp{p}_{qb}", tag="rcp")
                with nc.allow_low_precision(reason="denom recip"):
                    nc.vector.reciprocal(rcp[:], dnp[:])
                ds2 = dscrpool.tile([1, 2 * QB], F32R, name=f"ds2_{p}_{qb}", tag="ds2")
                dq.dma_start(ds2[0, :].rearrange("(pp f) -> pp f", f=FH), rcp[:])
                bc_sb = bcpool.tile([128, QB], F32R, name=f"bc{p}_{qb}", tag="bc")
                dq.dma_start(bc_sb[0:DK, :],
                             ds2[0:1, 0:QB].broadcast_to([DK, QB]))
                dq.dma_start(bc_sb[DK:2 * DK, :],
                             ds2[0:1, QB:2 * QB].broadcast_to([DK, QB]))
                nc.gpsimd.tensor_mul(cs_e, cs_e, bc_sb[0:DK, :])
                nc.gpsimd.tensor_mul(cs_o, cs_o, bc_sb[DK:2 * DK, :])

            # ---- out-projection (one stl = one 128-row output tile) ----
            def outproj_stl(qb, stl, defer_last=False):
                st = qb * FQ + stl
                halves = []
                for nck in range(DO // NCK):
                    ps = psc.tile([128, NCK], F32, name=f"op{st}_{nck}", tag=ftag())
                    tlast = n_ht - 1 if defer_last else n_ht
                    for t in range(tlast):
                        nc.tensor.matmul(
                            ps[:], ctx_stack[t][:, st * 128:(st + 1) * 128],
                            wo_t[:, t, nck * NCK:(nck + 1) * NCK],
                            start=(t == 0), stop=(t == n_ht - 1))
                    halves.append(ps)
                return st, halves

            def outproj_finish(st, halves):
                ev = oev.tile([128, DO], BF16, name=f"opev{st}", tag="ev")
                for nck, ps in enumerate(halves):
                    t = n_ht - 1
                    nc.tensor.matmul(
                        ps[:], ctx_stack[t][:, st * 128:(st + 1) * 128],
                        wo_t[:, t, nck * NCK:(nck + 1) * NCK],
                        start=False, stop=True)
                for nck, ps in enumerate(halves):
                    nc.vector.tensor_copy(ev[:, nck * NCK:(nck + 1) * NCK], ps[:])
                nc.gpsimd.dma_start(out[st * 128:(st + 1) * 128, :], ev[:])

            def outproj(qb, stl):
                st, halves = outproj_stl(qb, stl, defer_last=True)
                outproj_finish(st, halves)

            # ---- head block: rows 0..QB-1 fully bf16 ----
            def head_block_pair(p):
                he, ho = 2 * p, 2 * p + 1
                ctx_e = pctx.tile([VA, QB], F32, tag="ctx_e")
                ctx_o = pctx.tile([VA, QB], F32, tag="ctx_o")
                exs = []
                for kt in range(NHB):
                    rl = kt * 128
                    sc = psc.tile([128, 2, QB], F32, name=f"hsc{p}_{kt}",
                                  tag=("scA" if kt % 2 == 0 else "scB"))
                    for par in range(2):
                        r0 = par * DK
                        nc.tensor.matmul(
                            sc[:, par, rl:QB],
                            kt_sb[p][r0:r0 + DK, kt * 128:(kt + 1) * 128],
                            qt_sb[p][r0:r0 + DK, rl:QB],
                            start=True, stop=True)
                    exb = exbpool.tile([128, 2, QB], BF16, name=f"exb{p}_{kt}",
                                       tag="exb")
                    nc.scalar.activation(exb[:, :, rl:QB], sc[:, :, rl:QB],
                                         AF.Exp, scale=scale)
                    nc.vector.tensor_mul(
                        exb[:, :, rl:rl + 128], exb[:, :, rl:rl + 128],
                        mask_sb[:, 128:256].rearrange("q c -> q () c")
                        .broadcast_to([128, 2, 128]))
                    exs.append((kt, rl, exb))
                for (kt, rl, exb) in exs:
                    for par, (cps, h) in enumerate(((ctx_e, he), (ctx_o, ho))):
                        nc.tensor.matmul(
                            cps[:, rl:QB], vb[kt][:, h * VA:(h + 1) * VA],
                            exb[:, par, rl:QB],
                            start=(kt == 0), stop=(kt == NHB - 1))
                evac_pair(p, 0, ctx_e, ctx_o)

            # ---- fp8 attention for q block qb >= 1 ----
            def av_ranges(j, qb):
                if j < 2 * qb:
                    return [(0, QB, j == 0, False)]
                if j == 2 * qb:
                    return [(0, HB, j == 0, True), (HB, QB, j == 0, False)]
                return [(HB, QB, False, True)]

            def do_av(j, exa, qb, ctx_e, ctx_o, he, ho):
                for par, (cps, h) in enumerate(((ctx_e, he), (ctx_o, ho))):
                    lhsT = v_pair[j][:, :, h * VAP:h * VAP + VA]
                    for (c0, c1, st_, sp_) in av_ranges(j, qb):
                        nc.tensor.matmul(
                            cps[:, c0:c1], lhsT, exa[:, par, :, c0:c1],
                            start=st_, stop=sp_, perf_mode=DR)

            def attn_pair(qb, p):
                he, ho = 2 * p, 2 * p + 1
                ctx_e = pctx.tile([VA, QB], F32, tag="ctx_e")
                ctx_o = pctx.tile([VA, QB], F32, tag="ctx_o")
                n_jp = 2 * (qb + 1)
                pend_av = []
                for j in range(n_jp):
                    exa = expool.tile([128, 2, 2, QB], FP8,
                                      name=f"exa{qb}_{p}_{j}", tag="exa")
                    rel0 = max(0, 2 * j * 128 - qb * QB)
                    for ki in range(2):
                        kt = 2 * j + ki
                        rl = max(0, kt * 128 - qb * QB)
                        sc = psc.tile([128, 2, QB], F32,
                                      name=f"sc{qb}_{p}_{kt}",
                                      tag=("scA" if kt % 2 == 0 else "scB"))
                        for par in range(2):
                            r0 = par * DK
                            nc.tensor.matmul(
                                sc[:, par, rl:QB],
                                kt_sb[p][r0:r0 + DK, kt * 128:(kt + 1) * 128],
                                qt_sb[p][r0:r0 + DK, qb * QB + rl:qb * QB + QB],
                                start=True, stop=True)
                        nc.scalar.activation(
                            exa[:, :, ki, rl:QB], sc[:, :, rl:QB],
                            AF.Exp, scale=scale)
                        if j >= 2 * qb:
                            if ki == 0:
                                nc.vector.tensor_mul(
                                    exa[:, :, 0, rl:rl + 128],
                                    exa[:, :, 0, rl:rl + 128],
                                    mask_sb[:, 128:256]
                                    .rearrange("q c -> q () c")
                                    .broadcast_to([128, 2, 128]))
                            else:
                                nc.vector.tensor_mul(
                                    exa[:, :, 1, rel0:rel0 + 256],
                                    exa[:, :, 1, rel0:rel0 + 256],
                                    mask_sb[:]
                                    .rearrange("q c -> q () c")
                                    .broadcast_to([128, 2, 256]))
                    pend_av.append((j, exa))
                    if len(pend_av) > 3:
                        aj, aexa = pend_av.pop(0)
                        do_av(aj, aexa, qb, ctx_e, ctx_o, he, ho)
                for (aj, aexa) in pend_av:
                    do_av(aj, aexa, qb, ctx_e, ctx_o, he, ho)
                evac_pair(p, qb, ctx_e, ctx_o, use_act=(p == n_pairs - 1))

            # ================= emission schedule =================
            # Phase A: bf16 mini-projections (head-block inputs)
            for st in range(NHB):
                proj_v_bf16(st)
            for p in range(n_pairs):
                proj_qk_bf16("k", p)
            for p in range(n_pairs):
                proj_qk_bf16("q", p)

            # Phase B: fp8 V for qb1 + first fp8 Q/K chunk, with the
            # bf16 head-block attention interleaved as ACT warmup
            proj_v_fp8(0); proj_v_fp8(1)
            head_block_pair(0)
            proj_v_fp8(2); proj_v_fp8(3)
            head_block_pair(1)
            proj_v_fp8(4); proj_v_fp8(5)
            head_block_pair(2)
            proj_v_fp8(6); proj_v_fp8(7)
            head_block_pair(3)
            for p in range(n_pairs):
                proj_qk_fp8("k", p, 1)
            for p in range(n_pairs):
                proj_qk_fp8("q", p, 1)

            bctx.close()

            # Phase C: fp8 attention with projection/outproj filler.
            # filler[qb] = list of closures run one-per-head-pair slot.
            filler = {
                1: [lambda st=st: proj_v_fp8(st) for st in range(8, 16)] +
                   [lambda p=p: proj_qk_fp8("k", p, 2) for p in range(n_pairs)] +
                   [lambda p=p: proj_qk_fp8("q", p, 2) for p in range(n_pairs)],
                2: [lambda s=s: outproj(0, s) for s in range(FQ)] +
                   [lambda p=p: proj_qk_fp8("k", p, 3) for p in range(n_pairs)] +
                   [lambda p=p: proj_qk_fp8("q", p, 3) for p in range(n_pairs)] +
                   [lambda s=s: outproj(1, s) for s in range(FQ)],
                3: [lambda s=s: outproj(2, s) for s in range(FQ)],
            }
            for qb in range(1, n_qb):
                fl = filler[qb]
                # spread filler units across the head-pair iterations,
                # front-loaded so nothing lands in the tail of qb3
                per = (len(fl) + 1) // 2 if qb == n_qb - 1 else \
                    (len(fl) + n_pairs - 1) // n_pairs
                for p in range(n_pairs):
                    attn_pair(qb, p)
                    for u in fl[p * per:(p + 1) * per]:
                        u()

            # Phase D: final out-projection.  Attention PSUM is released, so
            # all four output tiles build in parallel banks; the last
            # head-pair's accumulation is deferred until its normalize lands.
            pctx2.close()
            with tc.tile_pool(name="pod", bufs=1, space="PSUM") as pod:
                pend = []
                for stl in range(FQ):
                    st = (n_qb - 1) * FQ + stl
                    halves = []
                    for nck in range(DO // NCK):
                        ps = pod.tile([128, NCK], F32, name=f"od{stl}_{nck}",
                                      tag=f"od{stl}_{nck}")
                        for t in range(n_ht - 1):
                            nc.tensor.matmul(
                                ps[:], ctx_stack[t][:, st * 128:(st + 1) * 128],
                                wo_t[:, t, nck * NCK:(nck + 1) * NCK],
                                start=(t == 0), stop=False)
                        halves.append(ps)
                    pend.append((st, halves))
                for args in pend:
                    outproj_finish(*args)
        ctx.close()

    split_multiwaits(nc)
    return nc


def core_inputs(queries, keys, values, Wq, bq, Wk, bk, Wv, bv, Wo, core, n_cores=8,
                HPC=None):
    """Host-side shard prep for one core. core -> (batch, head-group)."""
    import ml_dtypes
    B = queries.shape[0]
    H = Wq.shape[0]
    groups = n_cores // B
    b, hg = core // groups, core % groups
    if HPC is None:
        HPC = H // groups
    h0 = hg * HPC
    DK = Wq.shape[2]
    QB = 512
    bf16 = ml_dtypes.bfloat16
    fp8 = ml_dtypes.float8_e4m3

    def q8(a):
        return np.clip(a, -240.0, 240.0).astype(fp8)

    def wsel(W):
        # [H, D, dk] -> [D, HPC*dk], head-major columns
        return np.ascontiguousarray(
            W[h0:h0 + HPC].transpose(1, 0, 2).reshape(W.shape[1], HPC * DK))

    def bpairs(bias, s):
        bsel = (bias[h0:h0 + HPC] * s).reshape(HPC // 2, 2 * DK)
        return np.ascontiguousarray(bsel.T).astype(np.float32)

    x = np.arange(128)[:, None]
    y = np.arange(128)[None, :]
    tri = (y - x >= 0).astype(np.float32)
    mask = np.concatenate([np.zeros((128, 128), np.float32), tri], axis=1)
    xq = np.ascontiguousarray(queries[b].T)
    xk = np.ascontiguousarray(keys[b].T)
    xv = np.ascontiguousarray(values[b].T)
    wq_, wk_, wv_ = wsel(Wq), wsel(Wk), wsel(Wv)
    D = xq.shape[0]
    S = xq.shape[1]
    HD = HPC * DK

    def packx8(x):
        # [D, S'] -> [128, D//256, 2, S'] (partition-major pairs)
        return np.ascontiguousarray(
            x.reshape(D // 256, 2, 128, x.shape[1]).transpose(2, 0, 1, 3))

    def packb(x):
        # [D, C] -> [128, D//128, C]
        return np.ascontiguousarray(
            x.reshape(D // 128, 128, x.shape[1]).transpose(1, 0, 2))

    wo_ = (np.ascontiguousarray(Wo[h0 * DK:(h0 + HPC) * DK, :]) / 32.0)
    return {
        "masks_in": mask.astype(bf16),
        "vones": np.ones((128, HPC, 1), fp8),
        "vonesb": np.ones((128, HPC, 1), bf16),
        "xqT": packx8(q8(xq[:, QB:])), "xkT": packx8(q8(xk[:, QB:])),
        "xvT": packx8(q8(xv)),
        "xqTb": packb(xq[:, :QB].astype(bf16)),
        "xkTb": packb(xk[:, :QB].astype(bf16)),
        "xvTb": packb(xv[:, :QB].astype(bf16)),
        "wq": packx8(q8(wq_ * 64)), "wk": packx8(q8(wk_ * 64)),
        "wv": packx8(q8(wv_ * 32)),
        "wqb": packb((wq_ * 64).astype(bf16)),
        "wkb": packb((wk_ * 64).astype(bf16)),
        "wvb": packb((wv_ * 32).astype(bf16)),
        "bqp": bpairs(bq, 64.0), "bkp": bpairs(bk, 64.0),
        "bqb": bpairs(bq, 64.0), "bkb": bpairs(bk, 64.0),
        "wo": np.ascontiguousarray(
            wo_.reshape(HD // 128, 128, wo_.shape[1]).transpose(1, 0, 2)
        ).astype(bf16),
    }


def assemble(results, B, n_cores, bias_total):
    """Sum head-group partials per batch and add the host-side bias."""
    groups = n_cores // B
    outs = []
    for b in range(B):
        acc = results[b * groups]["out"].astype(np.float64)
        for g in range(1, groups):
            acc = acc + results[b * groups + g]["out"].astype(np.float64)
        outs.append(acc + bias_total)
    return np.stack(outs).astype(np.float32)


# ---------------------------------------------------------------------------
# Harness entry point: full (unsharded) inputs -> full output.
# Shards batch (4) x head-halves (2) across the 8 NeuronCores, runs the Bass
# kernel via run_bass_kernel_spmd, then sums head-half partials per batch on
# the host (+ bias fold: out += bo + bv @ Wo, exact because attention rows
# sum to 1 after normalization).
# ---------------------------------------------------------------------------
_CACHE = {}


def kernel(**inputs):
    from concourse.bass_utils import run_bass_kernel_spmd

    queries = np.asarray(inputs["queries"], np.float32)
    keys = np.asarray(inputs["keys"], np.float32)
    values = np.asarray(inputs["values"], np.float32)
    Wq = np.asarray(inputs["Wq"], np.float32)
    bq = np.asarray(inputs["bq"], np.float32)
    Wk = np.asarray(inputs["Wk"], np.float32)
    bk = np.asarray(inputs["bk"], np.float32)
    Wv = np.asarray(inputs["Wv"], np.float32)
    bv = np.asarray(inputs["bv"], np.float32)
    Wo = np.asarray(inputs["Wo"], np.float32)
    bo = np.asarray(inputs["bo"], np.float32)

    B = queries.shape[0]
    n_cores = 8
    if "nc" not in _CACHE:
        _CACHE["nc"] = build()
    nc = _CACHE["nc"]
    in_maps = [core_inputs(queries, keys, values, Wq, bq, Wk, bk, Wv, bv, Wo,
                           core=c, n_cores=n_cores) for c in range(n_cores)]
    res = run_bass_kernel_spmd(nc, in_maps, list(range(n_cores)))
    bias_total = bo + bv.reshape(-1) @ Wo
    return assemble(res.results, B, n_cores, bias_total)
